# revision 5
# baseline (speedup 1.0000x reference)
# Trainium2 Bass kernel for nn_EnergyForceModel (GNN message passing, energy+forces).
# Sharding: atoms (and their i-sorted neighbor-pair segments) across 8 cores;
# small MLP params replicated; per-pair force grads returned and reduced on host.
import numpy as np
import ml_dtypes

import concourse.bacc as bacc
import concourse.mybir as mybir
import concourse.tile as tile
from concourse import bass
from concourse.bass_utils import run_bass_kernel_spmd

F32 = mybir.dt.float32
BF16 = mybir.dt.bfloat16
AF = mybir.ActivationFunctionType
ALU = mybir.AluOpType
AX = mybir.AxisListType

N, P, NB, NR, NS, H = 20000, 320000, 7, 5, 10, 512
RCUT, ETA = 6.0, 1.0
NCORES = 8
A = N // NCORES            # 2500 atoms per core
NBLK = 20                  # 128-atom blocks per core (2560 padded atoms)
APAD = NBLK * 128
NGRP = 5                   # MLP atom groups of 512
HPI = np.pi

# monomial order (chosen so the g_u backward op groups have affine column APs)
UU = [(0, 0), (1, 1), (2, 2), (0, 1), (1, 2), (0, 2)]           # xx yy zz xy yz xz
UUU = [(0, 0, 0), (1, 1, 1), (2, 2, 2), (0, 0, 1), (1, 1, 2), (2, 2, 0),
       (1, 1, 0), (2, 2, 1), (0, 0, 2), (0, 1, 2)]              # xxx..xyz
WMULT = np.array([1.0] + [1.0] * 3 + [1, 1, 1, 2, 2, 2] + [1, 1, 1, 3, 3, 3, 3, 3, 3, 6],
                 np.float32)  # [20]


def _monomials(u):
    Pn = u.shape[0]
    Y = np.empty((Pn, 20), np.float32)
    Y[:, 0] = 1.0
    Y[:, 1:4] = u
    for k, (a, b) in enumerate(UU):
        Y[:, 4 + k] = u[:, a] * u[:, b]
    for k, (a, b, c) in enumerate(UUU):
        Y[:, 10 + k] = u[:, a] * u[:, b] * u[:, c]
    return Y


def _host_prep(R, Z, idx, mu, W_rad, W1, b1, W2, b2, W3, b3, scale, shift, **_unused):
    i, j = np.asarray(idx[0]), np.asarray(idx[1])
    R = np.asarray(R, np.float32)
    # per (core, block) pair lists, sorted by atom within block
    core = i // A
    loc = i - core * A
    blk = loc // 128
    arel = loc % 128
    order = np.lexsort((arel, blk, core))
    i_s, j_s, core_s, blk_s, arel_s = i[order], j[order], core[order], blk[order], arel[order]
    counts = np.zeros((NCORES, NBLK), np.int64)
    np.add.at(counts, (core_s, blk_s), 1)
    C = int(np.ceil(counts.max() / 128))  # chunks per block (global, one NEFF)
    SL = C * 128                          # pair slots per block

    dr_all = (R[j_s] - R[i_s]).astype(np.float32)
    wsel_all = np.asarray(W_rad, np.float32)[np.asarray(Z)[j_s]].reshape(-1, 35)

    in_maps = []
    meta = []
    boff = np.zeros((NCORES, NBLK), np.int64)
    starts = np.zeros((NCORES, NBLK), np.int64)
    pos = 0
    for c in range(NCORES):
        for b in range(NBLK):
            starts[c, b] = pos
            pos += counts[c, b]

    iu, ju = np.triu_indices(NR)
    W1 = np.asarray(W1, np.float32)
    W1full = np.zeros((80, H), np.float32)
    W1full[0:5] = W1[0:5]
    for k in range(3):
        base = 5 + 25 * k
        for t, (r, s) in enumerate(zip(iu, ju)):
            W1full[base + r * 5 + s] = W1[5 + 15 * k + t]
    W2 = np.asarray(W2, np.float32)
    W3 = np.asarray(W3, np.float32)
    b1 = np.asarray(b1, np.float32)
    b2 = np.asarray(b2, np.float32)

    w1f_h = W1full.astype(ml_dtypes.bfloat16)                       # [80,512]
    w2sb_h = W2.reshape(4, 128, H).transpose(1, 0, 2).reshape(128, 4 * H).astype(ml_dtypes.bfloat16)
    w2t_h = W2.T.reshape(4, 128, H).transpose(1, 0, 2).reshape(128, 4 * H).astype(ml_dtypes.bfloat16)
    w1ft_h = W1full.T.reshape(4, 128, 80).transpose(1, 0, 2).reshape(128, 4 * 80).astype(ml_dtypes.bfloat16)
    w3_h = W3[:, 0].reshape(4, 128).T.astype(ml_dtypes.bfloat16).copy()     # [128,4]
    w3t_h = W3[:, 0].reshape(1, H).astype(ml_dtypes.bfloat16).copy()        # [1,512]
    b1c_h = b1.reshape(4, 128).T.astype(np.float32).copy()                  # [128,4]
    b2c_h = b2.reshape(4, 128).T.astype(np.float32).copy()
    cb_h = np.zeros((128, 2), np.float32); cb_h[:, 0] = 1e-6; cb_h[:, 1] = np.pi / 2
    mut_h = np.broadcast_to(np.asarray(mu, np.float32)[None, :], (128, NB)).copy()
    wm_h = np.broadcast_to(WMULT[None, :], (128, 20)).copy()
    eye_h = np.eye(128, dtype=ml_dtypes.bfloat16)

    scm = np.asarray(scale, np.float32)[np.asarray(Z)] * (np.asarray(Z) > 0)
    shm = np.asarray(shift, np.float32)[np.asarray(Z)] * (np.asarray(Z) > 0)

    for c in range(NCORES):
        dr_h = np.zeros((NBLK * 128, C * 3), np.float32)
        dr_h.reshape(NBLK, 128, C, 3)[:, :, :, 0] = 1.0   # pad slots: dr=(1,0,0)
        wsel_h = np.zeros((NBLK * 128, C * 35), np.float32)
        oh_h = np.zeros((NBLK * 128, C * 128), ml_dtypes.bfloat16)
        ohT_h = np.zeros((NBLK * 128, C * 128), ml_dtypes.bfloat16)
        slot_orig = np.full((NBLK, SL), -1, np.int64)     # slot -> sorted-pair row
        for b in range(NBLK):
            n = counts[c, b]
            s0 = starts[c, b]
            sl = np.arange(n)
            ss, cc = sl % 128, sl // 128
            ar = np.concatenate([arel_s[s0:s0 + n], np.full(SL - n, 127, np.int64)])
            slf = np.arange(SL)
            ssf, ccf = slf % 128, slf // 128
            drb = dr_h.reshape(NBLK, 128, C, 3)
            wsb = wsel_h.reshape(NBLK, 128, C, 35)
            drb[b, ss, cc] = dr_all[s0:s0 + n]
            wsb[b, ss, cc] = wsel_all[s0:s0 + n]
            ohb = oh_h.reshape(NBLK, 128, C, 128)
            ohTb = ohT_h.reshape(NBLK, 128, C, 128)
            ohb[b, ssf, ccf, ar] = 1.0
            ohTb[b, ar, ccf, ssf] = 1.0
            slot_orig[b, :n] = s0 + sl
        scT_h = np.zeros((1, APAD), ml_dtypes.bfloat16)
        scT_h[0, :A] = scm[c * A:(c + 1) * A].astype(ml_dtypes.bfloat16)
        in_maps.append(dict(dr_h=dr_h, wsel_h=wsel_h, oh_h=oh_h, ohT_h=ohT_h,
                            scT_h=scT_h, w1f_h=w1f_h, w2sb_h=w2sb_h, w2t_h=w2t_h,
                            w1ft_h=w1ft_h, w3_h=w3_h, w3t_h=w3t_h, b1c_h=b1c_h,
                            b2c_h=b2c_h, mut_h=mut_h, wm_h=wm_h, eye_h=eye_h, cb_h=cb_h))
        meta.append(dict(slot_orig=slot_orig))
    shared = dict(i_s=i_s, j_s=j_s, scm=scm, shm=shm,
                  b3=float(np.asarray(b3).reshape(-1)[0]))
    return in_maps, meta, shared, C


def build_nc(C):
    SL = C * 128
    nc = bacc.Bacc("TRN2", target_bir_lowering=False, debug=False, num_devices=NCORES)
    dr_d = nc.dram_tensor("dr_h", [NBLK * 128, C * 3], F32, kind="ExternalInput")
    wsel_d = nc.dram_tensor("wsel_h", [NBLK * 128, C * 35], F32, kind="ExternalInput")
    oh_d = nc.dram_tensor("oh_h", [NBLK * 128, C * 128], BF16, kind="ExternalInput")
    ohT_d = nc.dram_tensor("ohT_h", [NBLK * 128, C * 128], BF16, kind="ExternalInput")
    scT_d = nc.dram_tensor("scT_h", [1, APAD], BF16, kind="ExternalInput")
    w1f_d = nc.dram_tensor("w1f_h", [80, H], BF16, kind="ExternalInput")
    w2sb_d = nc.dram_tensor("w2sb_h", [128, 4 * H], BF16, kind="ExternalInput")
    w2t_d = nc.dram_tensor("w2t_h", [128, 4 * H], BF16, kind="ExternalInput")
    w1ft_d = nc.dram_tensor("w1ft_h", [128, 4 * 80], BF16, kind="ExternalInput")
    w3_d = nc.dram_tensor("w3_h", [128, 4], BF16, kind="ExternalInput")
    w3t_d = nc.dram_tensor("w3t_h", [1, H], BF16, kind="ExternalInput")
    b1c_d = nc.dram_tensor("b1c_h", [128, 4], F32, kind="ExternalInput")
    b2c_d = nc.dram_tensor("b2c_h", [128, 4], F32, kind="ExternalInput")
    mut_d = nc.dram_tensor("mut_h", [128, NB], F32, kind="ExternalInput")
    cb_d = nc.dram_tensor("cb_h", [128, 2], F32, kind="ExternalInput")
    wm_d = nc.dram_tensor("wm_h", [128, 20], F32, kind="ExternalInput")
    eye_d = nc.dram_tensor("eye_h", [128, 128], BF16, kind="ExternalInput")
    gdr_o = nc.dram_tensor("gdr_o", [NBLK * 128, C * 3], F32, kind="ExternalOutput")
    e_o = nc.dram_tensor("e_o", [1, APAD], F32, kind="ExternalOutput")

    with tile.TileContext(nc) as tc:
        with (tc.tile_pool(name="pconst", bufs=1) as pc,
              tc.tile_pool(name="ppair", bufs=5) as pp,
              tc.tile_pool(name="pwork", bufs=2) as pw,
              tc.tile_pool(name="pmlp", bufs=2) as pm,
              tc.tile_pool(name="ps1", bufs=3, space="PSUM") as ps1,
              tc.tile_pool(name="psgv", bufs=1, space="PSUM") as psgv):

            # constants
            w1f = pc.tile([80, H], BF16)
            w2sb = pc.tile([128, 4 * H], BF16)
            w2t = pc.tile([128, 4 * H], BF16)
            w1ft = pc.tile([128, 4 * 80], BF16)
            w3sb = pc.tile([128, 4], BF16)
            w3t = pc.tile([1, H], BF16)
            b1c = pc.tile([128, 4], F32)
            b2c = pc.tile([128, 4], F32)
            mut = pc.tile([128, NB], F32)
            cb = pc.tile([128, 2], F32)
            wm = pc.tile([128, 20], F32)
            eye = pc.tile([128, 128], BF16)
            scT = pc.tile([1, APAD], BF16)
            for t, d in [(w1f, w1f_d), (w2sb, w2sb_d), (w2t, w2t_d), (w1ft, w1ft_d),
                         (w3sb, w3_d), (w3t, w3t_d), (b1c, b1c_d), (b2c, b2c_d),
                         (mut, mut_d), (wm, wm_d), (eye, eye_d), (scT, scT_d),
                         (cb, cb_d)]:
                nc.sync.dma_start(t[:], d[:])
            gmT_all = pc.tile([80, APAD], BF16)

            TT = nc.vector.tensor_tensor
            TS = nc.vector.tensor_scalar
            STT = nc.vector.scalar_tensor_tensor
            RED = nc.vector.tensor_reduce
            ACT = nc.scalar.activation

            blk_state = {}

            def fwd_block(b):
                r0 = b * 128
                dr = pp.tile([128, C * 3], F32, tag="dr")
                wsel = pp.tile([128, C * 35], F32, tag="wsel")
                oh = pw.tile([128, C * 128], BF16, tag="oh")
                nc.sync.dma_start(dr[:], dr_d[r0:r0 + 128, :])
                nc.sync.dma_start(wsel[:], wsel_d[r0:r0 + 128, :])
                nc.sync.dma_start(oh[:], oh_d[r0:r0 + 128, :])
                dr3 = dr[:].rearrange("p (c k) -> p c k", c=C)

                sq = pw.tile([128, C * 3], F32, tag="sq")
                TT(sq[:].rearrange("p (c k) -> p c k", c=C), dr3, dr3, ALU.mult)
                d2 = pw.tile([128, C], F32, tag="d2")
                RED(d2[:], sq[:].rearrange("p (c k) -> p c k", c=C), AX.X, ALU.add)
                dist = pp.tile([128, C], F32, tag="dist")
                ACT(dist[:], d2[:], AF.Sqrt, bias=cb[:, 0:1])
                rdist = pp.tile([128, C], F32, tag="rdist")
                nc.vector.reciprocal(rdist[:], dist[:])
                unit = pp.tile([128, C * 3], F32, tag="unit")
                u3 = unit[:].rearrange("p (c k) -> p c k", c=C)
                TT(u3, dr3, rdist[:].unsqueeze(2).to_broadcast([128, C, 3]), ALU.mult)

                dc = pw.tile([128, C], F32, tag="dc")
                TS(dc[:], dist[:], RCUT, 0.0, ALU.min, ALU.add)
                sn = pw.tile([128, C], F32, tag="sn")
                ACT(sn[:], dc[:], AF.Sin, scale=float(HPI / (2 * RCUT)))
                cs = pw.tile([128, C], F32, tag="cs")
                ACT(cs[:], dc[:], AF.Sin, bias=cb[:, 1:2], scale=float(HPI / (2 * RCUT)))
                mask = pw.tile([128, C], F32, tag="mask")
                TS(mask[:], dist[:], RCUT, 0.0, ALU.is_lt, ALU.add)
                s2 = pw.tile([128, C], F32, tag="s2")
                TT(s2[:], sn[:], sn[:], ALU.mult)
                oms = pw.tile([128, C], F32, tag="oms")
                TS(oms[:], s2[:], -1.0, 1.0, ALU.mult, ALU.add)
                fc = pp.tile([128, C], F32, tag="fc")
                TT(fc[:], oms[:], mask[:], ALU.mult)
                sc_ = pw.tile([128, C], F32, tag="sc_")
                TT(sc_[:], sn[:], cs[:], ALU.mult)
                fcp = pp.tile([128, C], F32, tag="fcp")
                STT(fcp[:], sc_[:], float(-HPI / RCUT), mask[:], ALU.mult, ALU.mult)

                dmu = pp.tile([128, C * NB], F32, tag="dmu")
                dmu3 = dmu[:].rearrange("p (c b) -> p c b", c=C)
                TT(dmu3, dist[:].unsqueeze(2).to_broadcast([128, C, NB]),
                   mut[:].unsqueeze(1).to_broadcast([128, C, NB]), ALU.subtract)
                dm2 = pw.tile([128, C * NB], F32, tag="dm2")
                ACT(dm2[:], dmu[:], AF.Square)
                gb = pp.tile([128, C * NB], F32, tag="gb")
                ACT(gb[:], dm2[:], AF.Exp, scale=float(-ETA))
                basis = pw.tile([128, C * NB], F32, tag="basis")
                TT(basis[:].rearrange("p (c b) -> p c b", c=C),
                   gb[:].rearrange("p (c b) -> p c b", c=C),
                   fc[:].unsqueeze(2).to_broadcast([128, C, NB]), ALU.mult)

                wsel4 = wsel[:].rearrange("p (c r b) -> p c r b", c=C, r=5)
                prodw = pw.tile([128, C * 35], F32, tag="prodw")
                TT(prodw[:].rearrange("p (c r b) -> p c r b", c=C, r=5), wsel4,
                   basis[:].rearrange("p (c b) -> p c b", c=C).unsqueeze(2).to_broadcast([128, C, 5, NB]),
                   ALU.mult)
                radial = pp.tile([128, C * 5], F32, tag="radial")
                RED(radial[:].rearrange("p (c r) -> p c r", c=C),
                    prodw[:].rearrange("p (c r b) -> p c r b", c=C, r=5), AX.X, ALU.add)

                # monomials Y [128, C, 20]
                Y = pp.tile([128, C * 20], F32, tag="Y")
                Y3 = Y[:].rearrange("p (c m) -> p c m", c=C)
                nc.vector.memset(Y3[:, :, 0:1], 1.0)
                nc.vector.tensor_copy(Y3[:, :, 1:4], u3)
                TT(Y3[:, :, 4:7], u3, u3, ALU.mult)                       # xx yy zz
                TT(Y3[:, :, 7:8], u3[:, :, 0:1], u3[:, :, 1:2], ALU.mult)  # xy
                TT(Y3[:, :, 8:9], u3[:, :, 1:2], u3[:, :, 2:3], ALU.mult)  # yz
                TT(Y3[:, :, 9:10], u3[:, :, 0:1], u3[:, :, 2:3], ALU.mult)  # xz
                TT(Y3[:, :, 10:13], Y3[:, :, 4:7], u3, ALU.mult)           # xxx yyy zzz
                TT(Y3[:, :, 13:14], Y3[:, :, 4:5], u3[:, :, 1:2], ALU.mult)  # xxy
                TT(Y3[:, :, 14:15], Y3[:, :, 5:6], u3[:, :, 2:3], ALU.mult)  # yyz
                TT(Y3[:, :, 15:16], Y3[:, :, 6:7], u3[:, :, 0:1], ALU.mult)  # zzx
                TT(Y3[:, :, 16:17], Y3[:, :, 5:6], u3[:, :, 0:1], ALU.mult)  # yyx
                TT(Y3[:, :, 17:18], Y3[:, :, 6:7], u3[:, :, 1:2], ALU.mult)  # zzy
                TT(Y3[:, :, 18:19], Y3[:, :, 4:5], u3[:, :, 2:3], ALU.mult)  # xxz
                TT(Y3[:, :, 19:20], Y3[:, :, 7:8], u3[:, :, 2:3], ALU.mult)  # xyz

                vals = pw.tile([128, C * 100], BF16, tag="vals")
                TT(vals[:].rearrange("p (c r m) -> p c r m", c=C, r=5),
                   radial[:].rearrange("p (c r) -> p c r", c=C).unsqueeze(3).to_broadcast([128, C, 5, 20]),
                   Y3.unsqueeze(2).to_broadcast([128, C, 5, 20]), ALU.mult)

                M_ps = ps1.tile([128, 100], F32, space="PSUM", tag="ps1")
                for c in range(C):
                    nc.tensor.matmul(out=M_ps[:], lhsT=oh[:, c * 128:(c + 1) * 128],
                                     rhs=vals[:, c * 100:(c + 1) * 100],
                                     start=(c == 0), stop=(c == C - 1))
                M_sb = pp.tile([128, 100], F32, tag="M_sb")
                nc.scalar.copy(M_sb[:], M_ps[:])
                Mt = pp.tile([128, 100], F32, tag="Mt")
                M3v = M_sb[:].rearrange("p (r m) -> p r m", r=5)
                TT(Mt[:].rearrange("p (r m) -> p r m", r=5), M3v,
                   wm[:].unsqueeze(1).to_broadcast([128, 5, 20]), ALU.mult)

                gm_bf = pw.tile([128, 80], BF16, tag="gm_bf")
                nc.vector.tensor_copy(gm_bf[:, 0:5], M3v[:, :, 0])
                Mt3 = Mt[:].rearrange("p (r m) -> p r m", r=5)
                with nc.allow_low_precision(reason="bf16 gm features"):
                    for k, (lo, ln) in enumerate([(1, 3), (4, 6), (10, 10)]):
                        pk = pw.tile([128, 25 * ln], F32, tag=f"pk{k}")
                        TT(pk[:].rearrange("p (r s m) -> p r s m", r=5, s=5),
                           M3v[:, :, lo:lo + ln].unsqueeze(2).to_broadcast([128, 5, 5, ln]),
                           Mt3[:, :, lo:lo + ln].unsqueeze(1).to_broadcast([128, 5, 5, ln]),
                           ALU.mult)
                        RED(gm_bf[:, 5 + 25 * k: 5 + 25 * (k + 1)].rearrange("p (r s) -> p r s", r=5),
                            pk[:].rearrange("p (r s m) -> p r s m", r=5, s=5), AX.X, ALU.add)
                tp_ps = ps1.tile([80, 128], BF16, space="PSUM", tag="ps1")
                nc.tensor.transpose(tp_ps[:], gm_bf[:], eye[:])
                nc.scalar.copy(gmT_all[:, b * 128:(b + 1) * 128], tp_ps[:])
                blk_state[b] = dict(unit=unit, rdist=rdist, fc=fc, fcp=fcp, gb=gb,
                                    dmu=dmu, wsel=wsel, radial=radial, Y=Y, Mt=Mt, dist=dist)

            def mlp_group(g):
                a0 = g * 512
                gmT = gmT_all[:, a0:a0 + 512]
                z1t = pm.tile([128, 4 * H], BF16, tag="z1t")
                h1t = pm.tile([128, 4 * H], BF16, tag="h1t")
                for m in range(4):
                    zp = ps1.tile([128, H], F32, space="PSUM", tag="ps1")
                    nc.tensor.matmul(out=zp[:], lhsT=w1f[:, m * 128:(m + 1) * 128],
                                     rhs=gmT, start=True, stop=True)
                    ACT(z1t[:, m * H:(m + 1) * H], zp[:], AF.Identity, bias=b1c[:, m:m + 1])
                    ACT(h1t[:, m * H:(m + 1) * H], zp[:], AF.Silu, bias=b1c[:, m:m + 1])
                z2t = pm.tile([128, 4 * H], BF16, tag="z2t")
                h2t = pm.tile([128, 4 * H], BF16, tag="h2t")
                for m in range(4):
                    zp = ps1.tile([128, H], F32, space="PSUM", tag="ps1")
                    for k in range(4):
                        nc.tensor.matmul(out=zp[:], lhsT=w2sb[:, k * H + m * 128: k * H + (m + 1) * 128],
                                         rhs=h1t[:, k * H:(k + 1) * H],
                                         start=(k == 0), stop=(k == 3))
                    ACT(z2t[:, m * H:(m + 1) * H], zp[:], AF.Identity, bias=b2c[:, m:m + 1])
                    ACT(h2t[:, m * H:(m + 1) * H], zp[:], AF.Silu, bias=b2c[:, m:m + 1])
                ep = ps1.tile([1, H], F32, space="PSUM", tag="ps1")
                for k in range(4):
                    nc.tensor.matmul(out=ep[:], lhsT=w3sb[:, k:k + 1],
                                     rhs=h2t[:, k * H:(k + 1) * H],
                                     start=(k == 0), stop=(k == 3))
                e_sb = pm.tile([1, H], F32, tag="e_sb")
                nc.scalar.copy(e_sb[:], ep[:])
                nc.sync.dma_start(e_o[0:1, a0:a0 + 512], e_sb[:])
                # backward
                gz2t = pm.tile([128, 4 * H], BF16, tag="gz2t")
                for m in range(4):
                    gp = ps1.tile([128, H], F32, space="PSUM", tag="ps1")
                    nc.tensor.matmul(out=gp[:], lhsT=w3t[:, m * 128:(m + 1) * 128],
                                     rhs=scT[:, a0:a0 + 512], start=True, stop=True)
                    dsw = pm.tile([128, H], BF16, tag="dsw")
                    ACT(dsw[:], z2t[:, m * H:(m + 1) * H], AF.Derivative_silu)
                    TT(gz2t[:, m * H:(m + 1) * H], gp[:], dsw[:], ALU.mult)
                gz1t = pm.tile([128, 4 * H], BF16, tag="gz1t")
                for m in range(4):
                    gp = ps1.tile([128, H], F32, space="PSUM", tag="ps1")
                    for k in range(4):
                        nc.tensor.matmul(out=gp[:], lhsT=w2t[:, k * H + m * 128: k * H + (m + 1) * 128],
                                         rhs=gz2t[:, k * H:(k + 1) * H],
                                         start=(k == 0), stop=(k == 3))
                    dsw = pm.tile([128, H], BF16, tag="dsw")
                    ACT(dsw[:], z1t[:, m * H:(m + 1) * H], AF.Derivative_silu)
                    TT(gz1t[:, m * H:(m + 1) * H], gp[:], dsw[:], ALU.mult)
                ggp = ps1.tile([80, 512], F32, space="PSUM", tag="ps1")
                for k in range(4):
                    nc.tensor.matmul(out=ggp[:], lhsT=w1ft[:, k * 80:(k + 1) * 80],
                                     rhs=gz1t[:, k * H:(k + 1) * H],
                                     start=(k == 0), stop=(k == 3))
                ggmt = pm.tile([80, 512], BF16, tag="ggmt")
                nc.scalar.copy(ggmt[:], ggp[:])
                return ggmt

            def bwd_block(b, ggmt, bi):
                st = blk_state.pop(b)
                unit, rdist, fc, fcp = st["unit"], st["rdist"], st["fc"], st["fcp"]
                gb, dmu, wsel, radial, Y, Mt = st["gb"], st["dmu"], st["wsel"], st["radial"], st["Y"], st["Mt"]
                u3 = unit[:].rearrange("p (c k) -> p c k", c=C)
                Y3 = Y[:].rearrange("p (c m) -> p c m", c=C)
                r0 = b * 128

                tpb = ps1.tile([128, 80], BF16, space="PSUM", tag="ps1")
                nc.tensor.transpose(tpb[:], ggmt[:, bi * 128:(bi + 1) * 128], eye[:80, :80])
                g_gm = pw.tile([128, 80], F32, tag="g_gm")
                nc.scalar.copy(g_gm[:], tpb[:])

                G = pw.tile([128, 75], F32, tag="G")
                gc = g_gm[:, 5:80].rearrange("p (k r s) -> p k r s", k=3, r=5)
                TT(G[:].rearrange("p (k r s) -> p k r s", k=3, r=5), gc,
                   gc.transpose([0, 1, 3, 2]), ALU.add)
                gM = pw.tile([128, 100], F32, tag="gM")
                gM3 = gM[:].rearrange("p (r m) -> p r m", r=5)
                nc.vector.tensor_copy(gM3[:, :, 0], g_gm[:, 0:5])
                G4 = G[:].rearrange("p (k r s) -> p k r s", k=3, r=5)
                Mt3 = Mt[:].rearrange("p (r m) -> p r m", r=5)
                for k, (lo, ln) in enumerate([(1, 3), (4, 6), (10, 10)]):
                    pk = pw.tile([128, 25 * ln], F32, tag=f"bk{k}")
                    TT(pk[:].rearrange("p (r s m) -> p r s m", r=5, s=5),
                       G4[:, k].unsqueeze(3).to_broadcast([128, 5, 5, ln]),
                       Mt3[:, :, lo:lo + ln].unsqueeze(1).to_broadcast([128, 5, 5, ln]),
                       ALU.mult)
                    RED(gM3[:, :, lo:lo + ln],
                        pk[:].rearrange("p (r s m) -> p r m s", r=5, s=5), AX.X, ALU.add)
                gM_bf = pw.tile([128, 100], BF16, tag="gM_bf")
                nc.vector.tensor_copy(gM_bf[:], gM[:])

                ohT = pw.tile([128, C * 128], BF16, tag="ohT")
                nc.sync.dma_start(ohT[:], ohT_d[r0:r0 + 128, :])
                gv = psgv.tile([128, C * 128], F32, space="PSUM", tag="psgv")
                for c in range(C):
                    nc.tensor.matmul(out=gv[:, c * 128: c * 128 + 100], lhsT=ohT[:, c * 128:(c + 1) * 128],
                                     rhs=gM_bf[:], start=True, stop=True)
                gv4 = gv[:].rearrange("p (c q) -> p c q", c=C)[:, :, 0:100].rearrange(
                    "p c (r m) -> p c r m", r=5)

                prod = pw.tile([128, C * 100], F32, tag="prod")
                p4 = prod[:].rearrange("p (c r m) -> p c r m", c=C, r=5)
                TT(p4, gv4, Y3.unsqueeze(2).to_broadcast([128, C, 5, 20]), ALU.mult)
                g_rad = pw.tile([128, C * 5], F32, tag="g_rad")
                RED(g_rad[:].rearrange("p (c r) -> p c r", c=C), p4, AX.X, ALU.add)
                prod2 = pw.tile([128, C * 100], F32, tag="prod2")
                p24 = prod2[:].rearrange("p (c r m) -> p c r m", c=C, r=5)
                TT(p24, gv4,
                   radial[:].rearrange("p (c r) -> p c r", c=C).unsqueeze(3).to_broadcast([128, C, 5, 20]),
                   ALU.mult)
                gY = pw.tile([128, C * 20], F32, tag="gY")
                gY3 = gY[:].rearrange("p (c m) -> p c m", c=C)
                RED(gY3, prod2[:].rearrange("p (c r m) -> p c m r", c=C, r=5), AX.X, ALU.add)

                # g_u accumulation
                gu = pw.tile([128, C * 3], F32, tag="gu")
                gu3 = gu[:].rearrange("p (c k) -> p c k", c=C)
                tmp3 = pw.tile([128, C * 3], F32, tag="tmp3")
                t3 = tmp3[:].rearrange("p (c k) -> p c k", c=C)
                tmp2 = pw.tile([128, C * 2], F32, tag="tmp2")
                t2 = tmp2[:].rearrange("p (c k) -> p c k", c=C)
                tmp1 = pw.tile([128, C], F32, tag="tmp1")
                t1 = tmp1[:].unsqueeze(2)

                def acc3(src):
                    TT(gu3, gu3, src, ALU.add)

                nc.vector.tensor_copy(gu3, gY3[:, :, 1:4])
                STT(t3, gY3[:, :, 4:7], 2.0, u3, ALU.mult, ALU.mult)      # diag
                acc3(t3)
                TT(t2, gY3[:, :, 7:9], u3[:, :, 1:3], ALU.mult)           # xy,yz -> x,y
                TT(gu3[:, :, 0:2], gu3[:, :, 0:2], t2, ALU.add)
                TT(t2, gY3[:, :, 7:9], u3[:, :, 0:2], ALU.mult)           # xy,yz -> y,z
                TT(gu3[:, :, 1:3], gu3[:, :, 1:3], t2, ALU.add)
                TT(t1, gY3[:, :, 9:10], u3[:, :, 2:3], ALU.mult)          # xz -> x
                TT(gu3[:, :, 0:1], gu3[:, :, 0:1], t1, ALU.add)
                TT(t1, gY3[:, :, 9:10], u3[:, :, 0:1], ALU.mult)          # xz -> z
                TT(gu3[:, :, 2:3], gu3[:, :, 2:3], t1, ALU.add)
                STT(t3, gY3[:, :, 10:13], 3.0, Y3[:, :, 4:7], ALU.mult, ALU.mult)  # pures
                acc3(t3)
                # aab group1: xxy,yyz,zzx (13,14,15): main 2*gY*Y[xy,yz,xz]
                STT(t3, gY3[:, :, 13:16], 2.0, Y3[:, :, 7:10], ALU.mult, ALU.mult)
                acc3(t3)
                # partner: out(y,z) += gY(13,14)*Y(xx,yy); out(x) += gY(15)*Y(zz)
                TT(t2, gY3[:, :, 13:15], Y3[:, :, 4:6], ALU.mult)
                TT(gu3[:, :, 1:3], gu3[:, :, 1:3], t2, ALU.add)
                TT(t1, gY3[:, :, 15:16], Y3[:, :, 6:7], ALU.mult)
                TT(gu3[:, :, 0:1], gu3[:, :, 0:1], t1, ALU.add)
                # aab group2: yyx,zzy,xxz (16,17,18): main 2*gY*Y[xy,yz,xz] -> out (y,z,x)
                STT(t2, gY3[:, :, 16:18], 2.0, Y3[:, :, 7:9], ALU.mult, ALU.mult)
                TT(gu3[:, :, 1:3], gu3[:, :, 1:3], t2, ALU.add)
                STT(t1, gY3[:, :, 18:19], 2.0, Y3[:, :, 9:10], ALU.mult, ALU.mult)
                TT(gu3[:, :, 0:1], gu3[:, :, 0:1], t1, ALU.add)
                # partner2: out(x,y) += gY(16,17)*Y(yy,zz); out(z) += gY(18)*Y(xx)
                TT(t2, gY3[:, :, 16:18], Y3[:, :, 5:7], ALU.mult)
                TT(gu3[:, :, 0:2], gu3[:, :, 0:2], t2, ALU.add)
                TT(t1, gY3[:, :, 18:19], Y3[:, :, 4:5], ALU.mult)
                TT(gu3[:, :, 2:3], gu3[:, :, 2:3], t1, ALU.add)
                # xyz (19): out(x,y) += gY*Y(yz,xz); out(z) += gY*Y(xy)
                TT(t2, gY3[:, :, 19:20].to_broadcast([128, C, 2]), Y3[:, :, 8:10], ALU.mult)
                TT(gu3[:, :, 0:2], gu3[:, :, 0:2], t2, ALU.add)
                TT(t1, gY3[:, :, 19:20], Y3[:, :, 7:8], ALU.mult)
                TT(gu3[:, :, 2:3], gu3[:, :, 2:3], t1, ALU.add)

                # g_basis and g_dist chain
                wsel4 = wsel[:].rearrange("p (c r b) -> p c b r", c=C, r=5)
                prwb = pw.tile([128, C * 35], F32, tag="prwb")
                TT(prwb[:].rearrange("p (c b r) -> p c b r", c=C, b=NB), wsel4,
                   g_rad[:].rearrange("p (c r) -> p c r", c=C).unsqueeze(2).to_broadcast([128, C, NB, 5]),
                   ALU.mult)
                gbas = pw.tile([128, C * NB], F32, tag="gbas")
                RED(gbas[:].rearrange("p (c b) -> p c b", c=C),
                    prwb[:].rearrange("p (c b r) -> p c b r", c=C, b=NB), AX.X, ALU.add)
                Av = pw.tile([128, C * NB], F32, tag="Av")
                TT(Av[:], gbas[:], gb[:], ALU.mult)
                Ar = pw.tile([128, C], F32, tag="Ar")
                RED(Ar[:], Av[:].rearrange("p (c b) -> p c b", c=C), AX.X, ALU.add)
                Bv = pw.tile([128, C * NB], F32, tag="Bv")
                TT(Bv[:], Av[:], dmu[:], ALU.mult)
                Br = pw.tile([128, C], F32, tag="Br")
                RED(Br[:], Bv[:].rearrange("p (c b) -> p c b", c=C), AX.X, ALU.add)
                gd1 = pw.tile([128, C], F32, tag="gd1")
                STT(gd1[:], Br[:], float(-2 * ETA), fc[:], ALU.mult, ALU.mult)
                td = pw.tile([128, C], F32, tag="td")
                TT(td[:], Ar[:], fcp[:], ALU.mult)
                gd12 = pw.tile([128, C], F32, tag="gd12")
                TT(gd12[:], gd1[:], td[:], ALU.add)
                udp = pw.tile([128, C * 3], F32, tag="udp")
                TT(udp[:].rearrange("p (c k) -> p c k", c=C), gu3, u3, ALU.mult)
                udot = pw.tile([128, C], F32, tag="udot")
                RED(udot[:], udp[:].rearrange("p (c k) -> p c k", c=C), AX.X, ALU.add)
                tu = pw.tile([128, C], F32, tag="tu")
                TT(tu[:], udot[:], rdist[:], ALU.mult)
                gdist = pw.tile([128, C], F32, tag="gdist")
                TT(gdist[:], gd12[:], tu[:], ALU.subtract)
                h1v = pw.tile([128, C * 3], F32, tag="h1v")
                TT(h1v[:].rearrange("p (c k) -> p c k", c=C), gu3,
                   rdist[:].unsqueeze(2).to_broadcast([128, C, 3]), ALU.mult)
                gdr = pw.tile([128, C * 3], F32, tag="gdr")
                g3 = gdr[:].rearrange("p (c k) -> p c k", c=C)
                TT(g3, u3, gdist[:].unsqueeze(2).to_broadcast([128, C, 3]), ALU.mult)
                TT(g3, g3, h1v[:].rearrange("p (c k) -> p c k", c=C), ALU.add)
                nc.sync.dma_start(gdr_o[r0:r0 + 128, :], gdr[:])

            for g in range(NGRP):
                for b in range(g * 4, g * 4 + 4):
                    fwd_block(b)
                ggmt = mlp_group(g)
                for bi, b in enumerate(range(g * 4, g * 4 + 4)):
                    bwd_block(b, ggmt, bi)

    nc.compile()
    return nc


def kernel(**inputs):
    in_maps, meta, shared, C = _host_prep(**inputs)
    nc = build_nc(C)
    res = run_bass_kernel_spmd(nc, in_maps, core_ids=list(range(NCORES)))
    i_s, j_s = shared["i_s"], shared["j_s"]
    scm, shm, b3 = shared["scm"], shared["shm"], shared["b3"]
    gdr_all = np.zeros((len(i_s), 3), np.float32)
    E = 0.0
    for c in range(NCORES):
        e_raw = res.results[c]["e_o"][0]                  # [APAD]
        E += float((e_raw[:A] + b3) @ scm[c * A:(c + 1) * A]) + float(shm[c * A:(c + 1) * A].sum())
        gdr = res.results[c]["gdr_o"].reshape(NBLK, 128, C, 3).transpose(0, 2, 1, 3).reshape(NBLK, C * 128, 3)
        so = meta[c]["slot_orig"]                          # [NBLK, SL]
        valid = so >= 0
        gdr_all[so[valid]] = gdr[valid]
    F = np.zeros((N, 3), np.float64)
    for k in range(3):
        F[:, k] = np.bincount(i_s, weights=gdr_all[:, k], minlength=N) \
                - np.bincount(j_s, weights=gdr_all[:, k], minlength=N)
    return np.float32(E), F.astype(np.float32)


# revision 18
# speedup vs baseline: 171.8838x; 171.8838x over previous
# Trainium2 Bass kernel for nn_EnergyForceModel (GNN message passing, energy+forces).
# Sharding: atoms (and their i-sorted neighbor-pair segments) across 8 cores;
# small MLP params replicated; per-pair force grads returned and reduced on host.
import numpy as np
import ml_dtypes

import concourse.bacc as bacc
import concourse.mybir as mybir
import concourse.tile as tile
from concourse import bass
from concourse.bass_utils import run_bass_kernel_spmd

F32 = mybir.dt.float32
BF16 = mybir.dt.bfloat16
AF = mybir.ActivationFunctionType
ALU = mybir.AluOpType
AX = mybir.AxisListType

N, P, NB, NR, NS, H = 20000, 320000, 7, 5, 10, 512
RCUT, ETA = 6.0, 1.0
NCORES = 8
A = N // NCORES            # 2500 atoms per core
NBLK = 20                  # 128-atom blocks per core (2560 padded atoms)
APAD = NBLK * 128
NGRP = 5                   # MLP atom groups of 512
GP_BLOCKS = {1, 3, 5, 9, 11, 13, 17}   # blocks whose elementwise chains run on GPSIMD
HPI = np.pi

# monomial order (chosen so the g_u backward op groups have affine column APs)
UU = [(0, 0), (1, 1), (2, 2), (0, 1), (1, 2), (0, 2)]           # xx yy zz xy yz xz
UUU = [(0, 0, 0), (1, 1, 1), (2, 2, 2), (0, 0, 1), (1, 1, 2), (2, 2, 0),
       (1, 1, 0), (2, 2, 1), (0, 0, 2), (0, 1, 2)]              # xxx..xyz
WMULT = np.array([1.0] + [1.0] * 3 + [1, 1, 1, 2, 2, 2] + [1, 1, 1, 3, 3, 3, 3, 3, 3, 6],
                 np.float32)  # [20]


def _monomials(u):
    Pn = u.shape[0]
    Y = np.empty((Pn, 20), np.float32)
    Y[:, 0] = 1.0
    Y[:, 1:4] = u
    for k, (a, b) in enumerate(UU):
        Y[:, 4 + k] = u[:, a] * u[:, b]
    for k, (a, b, c) in enumerate(UUU):
        Y[:, 10 + k] = u[:, a] * u[:, b] * u[:, c]
    return Y


def _host_prep(R, Z, idx, mu, W_rad, W1, b1, W2, b2, W3, b3, scale, shift, **_unused):
    i, j = np.asarray(idx[0]), np.asarray(idx[1])
    R = np.asarray(R, np.float32)
    # per (core, block) pair lists, sorted by atom within block
    core = i // A
    loc = i - core * A
    blk = loc // 128
    arel = loc % 128
    order = np.lexsort((arel, blk, core))
    i_s, j_s, core_s, blk_s, arel_s = i[order], j[order], core[order], blk[order], arel[order]
    counts = np.zeros((NCORES, NBLK), np.int64)
    np.add.at(counts, (core_s, blk_s), 1)
    C = int(np.ceil(counts.max() / 128))  # chunks per block (global, one NEFF)
    SL = C * 128                          # pair slots per block

    dr_all = (R[j_s] - R[i_s]).astype(np.float32)
    wsel_all = np.asarray(W_rad, np.float32)[np.asarray(Z)[j_s]].reshape(-1, 35)

    in_maps = []
    meta = []
    boff = np.zeros((NCORES, NBLK), np.int64)
    starts = np.zeros((NCORES, NBLK), np.int64)
    pos = 0
    for c in range(NCORES):
        for b in range(NBLK):
            starts[c, b] = pos
            pos += counts[c, b]

    iu, ju = np.triu_indices(NR)
    W1 = np.asarray(W1, np.float32)
    W1full = np.zeros((80, H), np.float32)
    W1full[0:5] = W1[0:5]
    for k in range(3):
        base = 5 + 25 * k
        for t, (r, s) in enumerate(zip(iu, ju)):
            W1full[base + r * 5 + s] = W1[5 + 15 * k + t]
    W2 = np.asarray(W2, np.float32)
    W3 = np.asarray(W3, np.float32)
    b1 = np.asarray(b1, np.float32)
    b2 = np.asarray(b2, np.float32)

    w1f_h = W1full.astype(ml_dtypes.bfloat16)                       # [80,512]
    w2sb_h = W2.reshape(4, 128, H).transpose(1, 0, 2).reshape(128, 4 * H).astype(ml_dtypes.bfloat16)
    w2t_h = W2.T.reshape(4, 128, H).transpose(1, 0, 2).reshape(128, 4 * H).astype(ml_dtypes.bfloat16)
    w1ft_h = W1full.T.reshape(4, 128, 80).transpose(1, 0, 2).reshape(128, 4 * 80).astype(ml_dtypes.bfloat16)
    w3_h = W3[:, 0].reshape(4, 128).T.astype(ml_dtypes.bfloat16).copy()     # [128,4]
    w3t_h = W3[:, 0].reshape(1, H).astype(ml_dtypes.bfloat16).copy()        # [1,512]
    b1c_h = b1.reshape(4, 128).T.astype(np.float32).copy()                  # [128,4]
    b2c_h = b2.reshape(4, 128).T.astype(np.float32).copy()
    cb_h = np.zeros((128, 2), np.float32); cb_h[:, 0] = 1e-6; cb_h[:, 1] = np.pi / 2
    mut_h = np.broadcast_to(np.asarray(mu, np.float32)[None, :], (128, NB)).copy()
    wm_h = np.broadcast_to(WMULT[None, :], (128, 20)).copy()
    eye_h = np.eye(128, dtype=ml_dtypes.bfloat16)

    scm = np.asarray(scale, np.float32)[np.asarray(Z)] * (np.asarray(Z) > 0)
    shm = np.asarray(shift, np.float32)[np.asarray(Z)] * (np.asarray(Z) > 0)

    for c in range(NCORES):
        dr_h = np.zeros((NBLK * 128, C * 3), np.float32)
        dr_h.reshape(NBLK, 128, C, 3)[:, :, :, 0] = 1.0   # pad slots: dr=(1,0,0)
        wsel_h = np.zeros((NBLK * 128, C * 35), np.float32)
        oh_h = np.zeros((NBLK * 128, C * 128), ml_dtypes.bfloat16)
        ohT_h = np.zeros((NBLK * 128, C * 128), ml_dtypes.bfloat16)
        slot_orig = np.full((NBLK, SL), -1, np.int64)     # slot -> sorted-pair row
        for b in range(NBLK):
            n = counts[c, b]
            s0 = starts[c, b]
            sl = np.arange(n)
            ss, cc = sl % 128, sl // 128
            ar = np.concatenate([arel_s[s0:s0 + n], np.full(SL - n, 127, np.int64)])
            slf = np.arange(SL)
            ssf, ccf = slf % 128, slf // 128
            drb = dr_h.reshape(NBLK, 128, C, 3)
            wsb = wsel_h.reshape(NBLK, 128, C, 35)
            drb[b, ss, cc] = dr_all[s0:s0 + n]
            wsb[b, ss, cc] = wsel_all[s0:s0 + n]
            ohb = oh_h.reshape(NBLK, 128, C, 128)
            ohTb = ohT_h.reshape(NBLK, 128, C, 128)
            ohb[b, ssf, ccf, ar] = 1.0
            ohTb[b, ar, ccf, ssf] = 1.0
            slot_orig[b, :n] = s0 + sl
        scT_h = np.zeros((1, APAD), ml_dtypes.bfloat16)
        scT_h[0, :A] = scm[c * A:(c + 1) * A].astype(ml_dtypes.bfloat16)
        in_maps.append(dict(dr_h=dr_h, wsel_h=wsel_h, oh_h=oh_h, ohT_h=ohT_h,
                            scT_h=scT_h, w1f_h=w1f_h, w2sb_h=w2sb_h, w2t_h=w2t_h,
                            w1ft_h=w1ft_h, w3_h=w3_h, w3t_h=w3t_h, b1c_h=b1c_h,
                            b2c_h=b2c_h, mut_h=mut_h, wm_h=wm_h, eye_h=eye_h, cb_h=cb_h))
        meta.append(dict(slot_orig=slot_orig))
    shared = dict(i_s=i_s, j_s=j_s, scm=scm, shm=shm,
                  b3=float(np.asarray(b3).reshape(-1)[0]))
    return in_maps, meta, shared, C


def build_nc(C, phases='all'):
    SL = C * 128
    nc = bacc.Bacc("TRN2", target_bir_lowering=False, debug=False, num_devices=NCORES)
    dr_d = nc.dram_tensor("dr_h", [NBLK * 128, C * 3], F32, kind="ExternalInput")
    wsel_d = nc.dram_tensor("wsel_h", [NBLK * 128, C * 35], F32, kind="ExternalInput")
    oh_d = nc.dram_tensor("oh_h", [NBLK * 128, C * 128], BF16, kind="ExternalInput")
    ohT_d = nc.dram_tensor("ohT_h", [NBLK * 128, C * 128], BF16, kind="ExternalInput")
    scT_d = nc.dram_tensor("scT_h", [1, APAD], BF16, kind="ExternalInput")
    w1f_d = nc.dram_tensor("w1f_h", [80, H], BF16, kind="ExternalInput")
    w2sb_d = nc.dram_tensor("w2sb_h", [128, 4 * H], BF16, kind="ExternalInput")
    w2t_d = nc.dram_tensor("w2t_h", [128, 4 * H], BF16, kind="ExternalInput")
    w1ft_d = nc.dram_tensor("w1ft_h", [128, 4 * 80], BF16, kind="ExternalInput")
    w3_d = nc.dram_tensor("w3_h", [128, 4], BF16, kind="ExternalInput")
    w3t_d = nc.dram_tensor("w3t_h", [1, H], BF16, kind="ExternalInput")
    b1c_d = nc.dram_tensor("b1c_h", [128, 4], F32, kind="ExternalInput")
    b2c_d = nc.dram_tensor("b2c_h", [128, 4], F32, kind="ExternalInput")
    mut_d = nc.dram_tensor("mut_h", [128, NB], F32, kind="ExternalInput")
    cb_d = nc.dram_tensor("cb_h", [128, 2], F32, kind="ExternalInput")
    wm_d = nc.dram_tensor("wm_h", [128, 20], F32, kind="ExternalInput")
    eye_d = nc.dram_tensor("eye_h", [128, 128], BF16, kind="ExternalInput")
    gdr_o = nc.dram_tensor("gdr_o", [NBLK * 128, C * 3], F32, kind="ExternalOutput")
    e_o = nc.dram_tensor("e_o", [1, APAD], F32, kind="ExternalOutput")

    with tile.TileContext(nc) as tc:
        with (tc.tile_pool(name="pconst", bufs=1) as pc,
              tc.tile_pool(name="ppair", bufs=5) as pp,
              tc.tile_pool(name="pwork", bufs=3) as pw,
              tc.tile_pool(name="pbig", bufs=2) as pb,
              tc.tile_pool(name="pmlp", bufs=1) as pm,
              tc.tile_pool(name="ps1", bufs=3, space="PSUM") as ps1,
              tc.tile_pool(name="psgv", bufs=2, space="PSUM") as psgv):

            # constants
            w1f = pc.tile([80, H], BF16)
            w2sb = pc.tile([128, 4 * H], BF16)
            w2t = pc.tile([128, 4 * H], BF16)
            w1ft = pc.tile([128, 4 * 80], BF16)
            w3sb = pc.tile([128, 4], BF16)
            w3t = pc.tile([1, H], BF16)
            b1c = pc.tile([128, 4], F32)
            b2c = pc.tile([128, 4], F32)
            mut = pc.tile([128, NB], F32)
            cb = pc.tile([128, 2], F32)
            wm = pc.tile([128, 20], F32)
            eye = pc.tile([128, 128], BF16)
            scT = pc.tile([1, APAD], BF16)
            for t, d in [(w1f, w1f_d), (w2sb, w2sb_d), (w2t, w2t_d), (w1ft, w1ft_d),
                         (w3sb, w3_d), (w3t, w3t_d), (b1c, b1c_d), (b2c, b2c_d),
                         (mut, mut_d), (wm, wm_d), (eye, eye_d), (scT, scT_d),
                         (cb, cb_d)]:
                nc.sync.dma_start(t[:], d[:])
            gmT_all = pc.tile([80, APAD], BF16)

            RED = nc.vector.tensor_reduce
            ACT = nc.scalar.activation

            blk_state = {}

            def fwd_block(b):
                ve = nc.gpsimd if b in GP_BLOCKS else nc.vector
                TT = ve.tensor_tensor
                TS = nc.vector.tensor_scalar
                STT = nc.vector.scalar_tensor_tensor
                r0 = b * 128
                dr = pp.tile([128, C * 3], F32, tag="dr")
                wsel = pp.tile([128, C * 35], F32, tag="wsel")
                oh = pb.tile([128, C * 128], BF16, tag="oh")
                nc.scalar.dma_start(dr[:], dr_d[r0:r0 + 128, :])
                nc.scalar.dma_start(wsel[:], wsel_d[r0:r0 + 128, :])
                nc.sync.dma_start(oh[:], oh_d[r0:r0 + 128, :])
                dr3 = dr[:].rearrange("p (c k) -> p c k", c=C)

                sq = pw.tile([128, C * 3], F32, tag="sq")
                TT(sq[:].rearrange("p (c k) -> p c k", c=C), dr3, dr3, ALU.mult)
                d2 = pw.tile([128, C], F32, tag="d2")
                RED(d2[:], sq[:].rearrange("p (c k) -> p c k", c=C), AX.X, ALU.add)
                dist = pp.tile([128, C], F32, tag="dist")
                ACT(dist[:], d2[:], AF.Sqrt, bias=cb[:, 0:1])
                rdist = pp.tile([128, C], F32, tag="rdist")
                nc.vector.reciprocal(rdist[:], dist[:])
                unit = pp.tile([128, C * 3], F32, tag="unit")
                u3 = unit[:].rearrange("p (c k) -> p c k", c=C)
                TT(u3, dr3, rdist[:].unsqueeze(2).to_broadcast([128, C, 3]), ALU.mult)

                dc = pw.tile([128, C], F32, tag="dc")
                TS(dc[:], dist[:], RCUT, 0.0, ALU.min, ALU.add)
                sn = pw.tile([128, C], F32, tag="sn")
                ACT(sn[:], dc[:], AF.Sin, scale=float(HPI / (2 * RCUT)))
                cs = pw.tile([128, C], F32, tag="cs")
                ACT(cs[:], dc[:], AF.Sin, bias=cb[:, 1:2], scale=float(HPI / (2 * RCUT)))
                mask = pw.tile([128, C], F32, tag="mask")
                TS(mask[:], dist[:], RCUT, 0.0, ALU.is_lt, ALU.add)
                s2 = pw.tile([128, C], F32, tag="s2")
                TT(s2[:], sn[:], sn[:], ALU.mult)
                oms = pw.tile([128, C], F32, tag="oms")
                TS(oms[:], s2[:], -1.0, 1.0, ALU.mult, ALU.add)
                fc = pp.tile([128, C], F32, tag="fc")
                TT(fc[:], oms[:], mask[:], ALU.mult)
                sc_ = pw.tile([128, C], F32, tag="sc_")
                TT(sc_[:], sn[:], cs[:], ALU.mult)
                fcp = pp.tile([128, C], F32, tag="fcp")
                STT(fcp[:], sc_[:], float(-HPI / RCUT), mask[:], ALU.mult, ALU.mult)

                dmu = pp.tile([128, C * NB], F32, tag="dmu")
                dmu3 = dmu[:].rearrange("p (c b) -> p c b", c=C)
                TT(dmu3, dist[:].unsqueeze(2).to_broadcast([128, C, NB]),
                   mut[:].unsqueeze(1).to_broadcast([128, C, NB]), ALU.subtract)
                dm2 = pw.tile([128, C * NB], F32, tag="dm2")
                ACT(dm2[:], dmu[:], AF.Square)
                gb = pp.tile([128, C * NB], F32, tag="gb")
                ACT(gb[:], dm2[:], AF.Exp, scale=float(-ETA))
                basis = pw.tile([128, C * NB], F32, tag="basis")
                TT(basis[:].rearrange("p (c b) -> p c b", c=C),
                   gb[:].rearrange("p (c b) -> p c b", c=C),
                   fc[:].unsqueeze(2).to_broadcast([128, C, NB]), ALU.mult)

                wsel4 = wsel[:].rearrange("p (c r b) -> p c r b", c=C, r=5)
                prodw = pw.tile([128, C * 35], F32, tag="prodw")
                TT(prodw[:].rearrange("p (c r b) -> p c r b", c=C, r=5), wsel4,
                   basis[:].rearrange("p (c b) -> p c b", c=C).unsqueeze(2).to_broadcast([128, C, 5, NB]),
                   ALU.mult)
                radial = pp.tile([128, C * 5], F32, tag="radial")
                RED(radial[:].rearrange("p (c r) -> p c r", c=C),
                    prodw[:].rearrange("p (c r b) -> p c r b", c=C, r=5), AX.X, ALU.add)

                # monomials Y [128, C, 20]
                Y = pp.tile([128, C * 20], F32, tag="Y")
                Y3 = Y[:].rearrange("p (c m) -> p c m", c=C)
                nc.vector.memset(Y3[:, :, 0:1], 1.0)
                nc.vector.tensor_copy(Y3[:, :, 1:4], u3)
                TT(Y3[:, :, 4:7], u3, u3, ALU.mult)                       # xx yy zz
                TT(Y3[:, :, 7:8], u3[:, :, 0:1], u3[:, :, 1:2], ALU.mult)  # xy
                TT(Y3[:, :, 8:9], u3[:, :, 1:2], u3[:, :, 2:3], ALU.mult)  # yz
                TT(Y3[:, :, 9:10], u3[:, :, 0:1], u3[:, :, 2:3], ALU.mult)  # xz
                TT(Y3[:, :, 10:13], Y3[:, :, 4:7], u3, ALU.mult)           # xxx yyy zzz
                TT(Y3[:, :, 13:14], Y3[:, :, 4:5], u3[:, :, 1:2], ALU.mult)  # xxy
                TT(Y3[:, :, 14:15], Y3[:, :, 5:6], u3[:, :, 2:3], ALU.mult)  # yyz
                TT(Y3[:, :, 15:16], Y3[:, :, 6:7], u3[:, :, 0:1], ALU.mult)  # zzx
                TT(Y3[:, :, 16:17], Y3[:, :, 5:6], u3[:, :, 0:1], ALU.mult)  # yyx
                TT(Y3[:, :, 17:18], Y3[:, :, 6:7], u3[:, :, 1:2], ALU.mult)  # zzy
                TT(Y3[:, :, 18:19], Y3[:, :, 4:5], u3[:, :, 2:3], ALU.mult)  # xxz
                TT(Y3[:, :, 19:20], Y3[:, :, 7:8], u3[:, :, 2:3], ALU.mult)  # xyz

                vals = pb.tile([128, C * 100], BF16, tag="vals")
                TT(vals[:].rearrange("p (c r m) -> p c r m", c=C, r=5),
                   radial[:].rearrange("p (c r) -> p c r", c=C).unsqueeze(3).to_broadcast([128, C, 5, 20]),
                   Y3.unsqueeze(2).to_broadcast([128, C, 5, 20]), ALU.mult)

                M_ps = ps1.tile([128, 100], F32, space="PSUM", tag="ps1")
                for c in range(C):
                    nc.tensor.matmul(out=M_ps[:], lhsT=oh[:, c * 128:(c + 1) * 128],
                                     rhs=vals[:, c * 100:(c + 1) * 100],
                                     start=(c == 0), stop=(c == C - 1))
                M_sb = pp.tile([128, 100], F32, tag="M_sb")
                nc.scalar.copy(M_sb[:], M_ps[:])
                Mt = pp.tile([128, 100], F32, tag="Mt")
                M3v = M_sb[:].rearrange("p (r m) -> p r m", r=5)
                TT(Mt[:].rearrange("p (r m) -> p r m", r=5), M3v,
                   wm[:].unsqueeze(1).to_broadcast([128, 5, 20]), ALU.mult)

                gm_bf = pw.tile([128, 80], BF16, tag="gm_bf")
                nc.vector.tensor_copy(gm_bf[:, 0:5], M3v[:, :, 0])
                Mt3 = Mt[:].rearrange("p (r m) -> p r m", r=5)
                with nc.allow_low_precision(reason="bf16 gm features"):
                    for k, (lo, ln) in enumerate([(1, 3), (4, 6), (10, 10)]):
                        pk = pw.tile([128, 25 * ln], F32, tag=f"pk{k}")
                        TT(pk[:].rearrange("p (r s m) -> p r s m", r=5, s=5),
                           M3v[:, :, lo:lo + ln].unsqueeze(2).to_broadcast([128, 5, 5, ln]),
                           Mt3[:, :, lo:lo + ln].unsqueeze(1).to_broadcast([128, 5, 5, ln]),
                           ALU.mult)
                        RED(gm_bf[:, 5 + 25 * k: 5 + 25 * (k + 1)].rearrange("p (r s) -> p r s", r=5),
                            pk[:].rearrange("p (r s m) -> p r s m", r=5, s=5), AX.X, ALU.add)
                tp_ps = ps1.tile([80, 128], BF16, space="PSUM", tag="ps1")
                nc.tensor.transpose(tp_ps[:], gm_bf[:], eye[:])
                nc.scalar.copy(gmT_all[:, b * 128:(b + 1) * 128], tp_ps[:])
                blk_state[b] = dict(unit=unit, rdist=rdist, fc=fc, fcp=fcp, gb=gb,
                                    dmu=dmu, wsel=wsel, radial=radial, Y=Y, Mt=Mt, dist=dist)

            def mlp_group(g):
                TT = nc.vector.tensor_tensor
                a0 = g * 512
                gmT = gmT_all[:, a0:a0 + 512]
                z1t = pm.tile([128, 4 * H], BF16, tag="z1t")
                h1t = pm.tile([128, 4 * H], BF16, tag="h1t")
                for m in range(4):
                    zp = ps1.tile([128, H], F32, space="PSUM", tag="ps1")
                    nc.tensor.matmul(out=zp[:], lhsT=w1f[:, m * 128:(m + 1) * 128],
                                     rhs=gmT, start=True, stop=True)
                    ACT(z1t[:, m * H:(m + 1) * H], zp[:], AF.Identity, bias=b1c[:, m:m + 1])
                    ACT(h1t[:, m * H:(m + 1) * H], zp[:], AF.Silu, bias=b1c[:, m:m + 1])
                z2t = pm.tile([128, 4 * H], BF16, tag="z2t")
                h2t = pm.tile([128, 4 * H], BF16, tag="h2t")
                for m in range(4):
                    zp = ps1.tile([128, H], F32, space="PSUM", tag="ps1")
                    for k in range(4):
                        nc.tensor.matmul(out=zp[:], lhsT=w2sb[:, k * H + m * 128: k * H + (m + 1) * 128],
                                         rhs=h1t[:, k * H:(k + 1) * H],
                                         start=(k == 0), stop=(k == 3))
                    ACT(z2t[:, m * H:(m + 1) * H], zp[:], AF.Identity, bias=b2c[:, m:m + 1])
                    ACT(h2t[:, m * H:(m + 1) * H], zp[:], AF.Silu, bias=b2c[:, m:m + 1])
                ep = ps1.tile([1, H], F32, space="PSUM", tag="ps1")
                for k in range(4):
                    nc.tensor.matmul(out=ep[:], lhsT=w3sb[:, k:k + 1],
                                     rhs=h2t[:, k * H:(k + 1) * H],
                                     start=(k == 0), stop=(k == 3))
                e_sb = pm.tile([1, H], F32, tag="e_sb")
                nc.scalar.copy(e_sb[:], ep[:])
                nc.sync.dma_start(e_o[0:1, a0:a0 + 512], e_sb[:])
                # backward
                gz2t = pm.tile([128, 4 * H], BF16, tag="gz2t")
                for m in range(4):
                    gp = ps1.tile([128, H], F32, space="PSUM", tag="ps1")
                    nc.tensor.matmul(out=gp[:], lhsT=w3t[:, m * 128:(m + 1) * 128],
                                     rhs=scT[:, a0:a0 + 512], start=True, stop=True)
                    dsw = pm.tile([128, H], BF16, tag="dsw")
                    ACT(dsw[:], z2t[:, m * H:(m + 1) * H], AF.Derivative_silu)
                    TT(gz2t[:, m * H:(m + 1) * H], gp[:], dsw[:], ALU.mult)
                gz1t = pm.tile([128, 4 * H], BF16, tag="gz1t")
                for m in range(4):
                    gp = ps1.tile([128, H], F32, space="PSUM", tag="ps1")
                    for k in range(4):
                        nc.tensor.matmul(out=gp[:], lhsT=w2t[:, k * H + m * 128: k * H + (m + 1) * 128],
                                         rhs=gz2t[:, k * H:(k + 1) * H],
                                         start=(k == 0), stop=(k == 3))
                    dsw = pm.tile([128, H], BF16, tag="dsw")
                    ACT(dsw[:], z1t[:, m * H:(m + 1) * H], AF.Derivative_silu)
                    TT(gz1t[:, m * H:(m + 1) * H], gp[:], dsw[:], ALU.mult)
                ggp = ps1.tile([80, 512], F32, space="PSUM", tag="ps1")
                for k in range(4):
                    nc.tensor.matmul(out=ggp[:], lhsT=w1ft[:, k * 80:(k + 1) * 80],
                                     rhs=gz1t[:, k * H:(k + 1) * H],
                                     start=(k == 0), stop=(k == 3))
                ggmt = pm.tile([80, 512], BF16, tag="ggmt")
                nc.scalar.copy(ggmt[:], ggp[:])
                return ggmt

            def bwd_block(b, ggmt, bi):
                ve = nc.gpsimd if b in GP_BLOCKS else nc.vector
                TT = ve.tensor_tensor
                TS = nc.vector.tensor_scalar
                STT = nc.vector.scalar_tensor_tensor
                st = blk_state.pop(b)
                unit, rdist, fc, fcp = st["unit"], st["rdist"], st["fc"], st["fcp"]
                gb, dmu, wsel, radial, Y, Mt = st["gb"], st["dmu"], st["wsel"], st["radial"], st["Y"], st["Mt"]
                u3 = unit[:].rearrange("p (c k) -> p c k", c=C)
                Y3 = Y[:].rearrange("p (c m) -> p c m", c=C)
                r0 = b * 128

                tpb = ps1.tile([128, 80], BF16, space="PSUM", tag="ps1")
                nc.tensor.transpose(tpb[:], ggmt[:, bi * 128:(bi + 1) * 128], eye[:80, :80])
                g_gm = pw.tile([128, 80], F32, tag="g_gm")
                nc.scalar.copy(g_gm[:], tpb[:])

                G = pw.tile([128, 75], F32, tag="G")
                gc = g_gm[:, 5:80].rearrange("p (k r s) -> p k r s", k=3, r=5)
                TT(G[:].rearrange("p (k r s) -> p k r s", k=3, r=5), gc,
                   gc.transpose([0, 1, 3, 2]), ALU.add)
                gM = pw.tile([128, 100], F32, tag="gM")
                gM3 = gM[:].rearrange("p (r m) -> p r m", r=5)
                nc.vector.tensor_copy(gM3[:, :, 0], g_gm[:, 0:5])
                G4 = G[:].rearrange("p (k r s) -> p k r s", k=3, r=5)
                Mt3 = Mt[:].rearrange("p (r m) -> p r m", r=5)
                for k, (lo, ln) in enumerate([(1, 3), (4, 6), (10, 10)]):
                    pk = pw.tile([128, 25 * ln], F32, tag=f"bk{k}")
                    TT(pk[:].rearrange("p (r s m) -> p r s m", r=5, s=5),
                       G4[:, k].unsqueeze(3).to_broadcast([128, 5, 5, ln]),
                       Mt3[:, :, lo:lo + ln].unsqueeze(1).to_broadcast([128, 5, 5, ln]),
                       ALU.mult)
                    RED(gM3[:, :, lo:lo + ln],
                        pk[:].rearrange("p (r s m) -> p r m s", r=5, s=5), AX.X, ALU.add)
                gM_bf = pw.tile([128, 100], BF16, tag="gM_bf")
                nc.vector.tensor_copy(gM_bf[:], gM[:])

                ohT = pb.tile([128, C * 128], BF16, tag="ohT")
                nc.sync.dma_start(ohT[:], ohT_d[r0:r0 + 128, :])
                CH = C // 3
                g_rad = pw.tile([128, C * 5], F32, tag="g_rad")
                gY = pw.tile([128, C * 20], F32, tag="gY")
                gY3 = gY[:].rearrange("p (c m) -> p c m", c=C)
                for h in range(3):
                    c0 = h * CH
                    gv = psgv.tile([128, CH * 128], F32, space="PSUM", tag="psgv")
                    for c in range(CH):
                        nc.tensor.matmul(out=gv[:, c * 128: c * 128 + 100],
                                         lhsT=ohT[:, (c0 + c) * 128:(c0 + c + 1) * 128],
                                         rhs=gM_bf[:], start=True, stop=True)
                    if b in GP_BLOCKS:
                        gv_sb = pw.tile([128, CH * 100], F32, tag="gv_sb")
                        nc.scalar.copy(gv_sb[:].rearrange("p (c q) -> p c q", c=CH),
                                       gv[:].rearrange("p (c q) -> p c q", c=CH)[:, :, 0:100])
                        gv4 = gv_sb[:].rearrange("p (c r m) -> p c r m", c=CH, r=5)
                    else:
                        gv4 = gv[:].rearrange("p (c q) -> p c q", c=CH)[:, :, 0:100].rearrange(
                            "p c (r m) -> p c r m", r=5)
                    Yh = Y3[:, c0:c0 + CH]
                    radh = radial[:].rearrange("p (c r) -> p c r", c=C)[:, c0:c0 + CH]
                    prod = pw.tile([128, CH * 100], F32, tag="prod")
                    p4 = prod[:].rearrange("p (c r m) -> p c r m", c=CH, r=5)
                    TT(p4, gv4, Yh.unsqueeze(2).to_broadcast([128, CH, 5, 20]), ALU.mult)
                    RED(g_rad[:].rearrange("p (c r) -> p c r", c=C)[:, c0:c0 + CH], p4,
                        AX.X, ALU.add)
                    prod2 = pw.tile([128, CH * 100], F32, tag="prod2")
                    p24 = prod2[:].rearrange("p (c r m) -> p c r m", c=CH, r=5)
                    TT(p24, gv4, radh.unsqueeze(3).to_broadcast([128, CH, 5, 20]), ALU.mult)
                    RED(gY3[:, c0:c0 + CH],
                        prod2[:].rearrange("p (c r m) -> p c m r", c=CH, r=5), AX.X, ALU.add)

                # g_u: write 30 product terms into a [128, C, 3, 10] slot table,
                # then one reduce over the slot axis
                trm = pw.tile([128, C * 30], F32, tag="trm")
                T4 = trm[:].rearrange("p (c k t) -> p c k t", c=C, k=3)
                nc.vector.tensor_copy(T4[:, :, :, 0], gY3[:, :, 1:4])
                STT(T4[:, :, :, 1], gY3[:, :, 4:7], 2.0, u3, ALU.mult, ALU.mult)
                TT(T4[:, :, 0:2, 2], gY3[:, :, 7:9], u3[:, :, 1:3], ALU.mult)
                TT(T4[:, :, 2:3, 2], gY3[:, :, 9:10], u3[:, :, 0:1], ALU.mult)
                TT(T4[:, :, 1:3, 3], gY3[:, :, 7:9], u3[:, :, 0:2], ALU.mult)
                TT(T4[:, :, 0:1, 3], gY3[:, :, 9:10], u3[:, :, 2:3], ALU.mult)
                STT(T4[:, :, :, 4], gY3[:, :, 10:13], 3.0, Y3[:, :, 4:7], ALU.mult, ALU.mult)
                STT(T4[:, :, :, 5], gY3[:, :, 13:16], 2.0, Y3[:, :, 7:10], ALU.mult, ALU.mult)
                TT(T4[:, :, 1:3, 6], gY3[:, :, 13:15], Y3[:, :, 4:6], ALU.mult)
                TT(T4[:, :, 0:1, 6], gY3[:, :, 15:16], Y3[:, :, 6:7], ALU.mult)
                STT(T4[:, :, 1:3, 7], gY3[:, :, 16:18], 2.0, Y3[:, :, 7:9], ALU.mult, ALU.mult)
                STT(T4[:, :, 0:1, 7], gY3[:, :, 18:19], 2.0, Y3[:, :, 9:10], ALU.mult, ALU.mult)
                TT(T4[:, :, 0:2, 8], gY3[:, :, 16:18], Y3[:, :, 5:7], ALU.mult)
                TT(T4[:, :, 2:3, 8], gY3[:, :, 18:19], Y3[:, :, 4:5], ALU.mult)
                TT(T4[:, :, 0:2, 9], gY3[:, :, 19:20].to_broadcast([128, C, 2]), Y3[:, :, 8:10], ALU.mult)
                TT(T4[:, :, 2:3, 9], gY3[:, :, 19:20], Y3[:, :, 7:8], ALU.mult)
                gu = pw.tile([128, C * 3], F32, tag="gu")
                gu3 = gu[:].rearrange("p (c k) -> p c k", c=C)
                RED(gu3, T4, AX.X, ALU.add)

                # g_basis and g_dist chain
                wsel4 = wsel[:].rearrange("p (c r b) -> p c b r", c=C, r=5)
                prwb = pw.tile([128, C * 35], F32, tag="prwb")
                TT(prwb[:].rearrange("p (c b r) -> p c b r", c=C, b=NB), wsel4,
                   g_rad[:].rearrange("p (c r) -> p c r", c=C).unsqueeze(2).to_broadcast([128, C, NB, 5]),
                   ALU.mult)
                gbas = pw.tile([128, C * NB], F32, tag="gbas")
                RED(gbas[:].rearrange("p (c b) -> p c b", c=C),
                    prwb[:].rearrange("p (c b r) -> p c b r", c=C, b=NB), AX.X, ALU.add)
                Av = pw.tile([128, C * NB], F32, tag="Av")
                TT(Av[:], gbas[:], gb[:], ALU.mult)
                Ar = pw.tile([128, C], F32, tag="Ar")
                RED(Ar[:], Av[:].rearrange("p (c b) -> p c b", c=C), AX.X, ALU.add)
                Bv = pw.tile([128, C * NB], F32, tag="Bv")
                TT(Bv[:], Av[:], dmu[:], ALU.mult)
                Br = pw.tile([128, C], F32, tag="Br")
                RED(Br[:], Bv[:].rearrange("p (c b) -> p c b", c=C), AX.X, ALU.add)
                gd1 = pw.tile([128, C], F32, tag="gd1")
                STT(gd1[:], Br[:], float(-2 * ETA), fc[:], ALU.mult, ALU.mult)
                td = pw.tile([128, C], F32, tag="td")
                TT(td[:], Ar[:], fcp[:], ALU.mult)
                gd12 = pw.tile([128, C], F32, tag="gd12")
                TT(gd12[:], gd1[:], td[:], ALU.add)
                udp = pw.tile([128, C * 3], F32, tag="udp")
                TT(udp[:].rearrange("p (c k) -> p c k", c=C), gu3, u3, ALU.mult)
                udot = pw.tile([128, C], F32, tag="udot")
                RED(udot[:], udp[:].rearrange("p (c k) -> p c k", c=C), AX.X, ALU.add)
                tu = pw.tile([128, C], F32, tag="tu")
                TT(tu[:], udot[:], rdist[:], ALU.mult)
                gdist = pw.tile([128, C], F32, tag="gdist")
                TT(gdist[:], gd12[:], tu[:], ALU.subtract)
                h1v = pw.tile([128, C * 3], F32, tag="h1v")
                TT(h1v[:].rearrange("p (c k) -> p c k", c=C), gu3,
                   rdist[:].unsqueeze(2).to_broadcast([128, C, 3]), ALU.mult)
                gdr = pw.tile([128, C * 3], F32, tag="gdr")
                g3 = gdr[:].rearrange("p (c k) -> p c k", c=C)
                TT(g3, u3, gdist[:].unsqueeze(2).to_broadcast([128, C, 3]), ALU.mult)
                TT(g3, g3, h1v[:].rearrange("p (c k) -> p c k", c=C), ALU.add)
                nc.sync.dma_start(gdr_o[r0:r0 + 128, :], gdr[:])

            for g in range(NGRP):
                for b in range(g * 4, g * 4 + 4):
                    fwd_block(b)
                if phases == 'fwd':
                    continue
                ggmt = mlp_group(g)
                if phases == 'mlp':
                    continue
                for bi, b in enumerate(range(g * 4, g * 4 + 4)):
                    bwd_block(b, ggmt, bi)

    nc.compile()
    return nc


def kernel(**inputs):
    in_maps, meta, shared, C = _host_prep(**inputs)
    nc = build_nc(C)
    res = run_bass_kernel_spmd(nc, in_maps, core_ids=list(range(NCORES)))
    i_s, j_s = shared["i_s"], shared["j_s"]
    scm, shm, b3 = shared["scm"], shared["shm"], shared["b3"]
    gdr_all = np.zeros((len(i_s), 3), np.float32)
    E = 0.0
    for c in range(NCORES):
        e_raw = res.results[c]["e_o"][0]                  # [APAD]
        E += float((e_raw[:A] + b3) @ scm[c * A:(c + 1) * A]) + float(shm[c * A:(c + 1) * A].sum())
        gdr = res.results[c]["gdr_o"].reshape(NBLK, 128, C, 3).transpose(0, 2, 1, 3).reshape(NBLK, C * 128, 3)
        so = meta[c]["slot_orig"]                          # [NBLK, SL]
        valid = so >= 0
        gdr_all[so[valid]] = gdr[valid]
    F = np.zeros((N, 3), np.float64)
    for k in range(3):
        F[:, k] = np.bincount(i_s, weights=gdr_all[:, k], minlength=N) \
                - np.bincount(j_s, weights=gdr_all[:, k], minlength=N)
    return np.float32(E), F.astype(np.float32)
